# revision 1
# baseline (speedup 1.0000x reference)
"""Cross-attention kernel for Trainium2, sharded across 8 NeuronCores.

out = softmax(Q @ K^T) @ V with Q,K: [8192,512], V: [8192,512], fp32.

Sharding: query rows across the 8 cores (1024 rows each); K/V replicated.

Per-core algorithm (all in the S^T = K@Q^T layout so that no on-chip
transposes are needed):
  - Host pre-transposes Q and K and splits each element x into
    hi = round_f32r(x) (11-bit mantissa) and lo = x - hi.
  - S^T main term: Kh^T @ Qh as a float32r matmul (1 cycle/row on the PE
    vs 4 for fp32).
  - S^T cross terms (Kl@Qh + Kh@Ql, ~2^-12 of S): ONE fp8 DoubleRow
    matmul per d-chunk — stationary [d, 2, k] = [2^12*Kl | Kh], moving
    [d, 2, q] = [Qh | 2^12*Ql], contraction 256, 0.5 cycles/row. The
    result C = 2^12 * cross accumulates in its own PSUM bank.
  - exp(S - 100) = exp(S_hi - 100) * exp(2^-12 * C): two ACT activations
    (the 2^-12 is the activation's scale immediate) and one DVE multiply
    whose output dtype float32r rounds P for the P@V matmul.
    The constant bias -100 replaces the row max: scores are N(0, 512), so
    row maxes concentrate in [80, 115]; exp(S-100) neither overflows nor
    flushes an entire row to zero, and a constant shift cancels exactly
    in the normalization.
  - row sums (softmax denominators) come from tiny N=2 matmuls against a
    ones vector, accumulated in PSUM alongside the P@V accumulation.
  - P@V accumulates over all of K in PSUM, q-half at a time. PSUM banks:
    4 O + 1 rowsum + 2 S^T + 1 C = 8.
"""

import numpy as np

N_CORES = 8
NQ, NK, D, DV = 8192, 8192, 512, 512
QBLK = NQ // N_CORES          # 1024 query rows per core
QH = 512                      # q-half (moving-operand width for S^T matmul)
N_QH = QBLK // QH             # 2
KC = 512                      # k-chunk rows streamed per DMA
N_KC = NK // KC               # 16
KT_SUB = KC // 128            # 4 k-subtiles per chunk
DCH = D // 128                # 4 contraction chunks
QT_PER_H = QH // 128          # 4 q-tiles per half

CROSS_SCALE = 2048.0          # 2^11: fp16 hi-part residual scale

_compiled = None


def _round_f32r(x: np.ndarray) -> np.ndarray:
    """Round fp32 to f32r (11-bit mantissa, RTNE), matching the HW rounding."""
    b = np.ascontiguousarray(x).view(np.uint32)
    r = ((b >> np.uint32(12)) & np.uint32(1)) + np.uint32(0x7FF)
    return ((b + r) & np.uint32(0xFFFFF000)).view(np.float32)


def _build():
    import concourse.mybir as mybir
    import concourse.tile as tile
    from concourse import bacc

    f32 = mybir.dt.float32
    f32r = mybir.dt.float32r
    f8 = mybir.dt.float8e4
    f16 = mybir.dt.float16

    nc = bacc.Bacc("TRN2", target_bir_lowering=False, debug=False,
                   num_devices=N_CORES)

    qth_d = nc.dram_tensor("qth", [D, QBLK], f16, kind="ExternalInput").ap()
    qc8_d = nc.dram_tensor("qc8", [D, 2 * QBLK], f8, kind="ExternalInput").ap()
    kth_d = nc.dram_tensor("kth", [D, NK], f16, kind="ExternalInput").ap()
    kc8_d = nc.dram_tensor("kc8", [D, 2 * NK], f8, kind="ExternalInput").ap()
    v_d = nc.dram_tensor("v", [NK, DV], f32r, kind="ExternalInput").ap()
    ones_d = nc.dram_tensor("ones", [128, 2], f32r, kind="ExternalInput").ap()
    bias_d = nc.dram_tensor("bias", [128, 1], f32, kind="ExternalInput").ap()
    out_d = nc.dram_tensor("out", [QBLK, DV], f32, kind="ExternalOutput").ap()

    with tile.TileContext(nc) as tc:
        with tc.tile_pool(name="resident", bufs=1) as rpool, \
             tc.tile_pool(name="stream", bufs=3) as spool, \
             tc.tile_pool(name="etile", bufs=3) as epool, \
             tc.tile_pool(name="ptile", bufs=4) as ppool, \
             tc.tile_pool(name="outp", bufs=4) as opool, \
             tc.tile_pool(name="spsum", bufs=2, space="PSUM") as spsum, \
             tc.tile_pool(name="cpsum", bufs=2, space="PSUM") as cpsum, \
             tc.tile_pool(name="opsum", bufs=1, space="PSUM") as opsum:

            # Resident: Q^T hi as [128, DCH, QBLK]; fp8 cross pack as
            # [128, DCH, 2, QBLK]
            qth = rpool.tile([128, DCH * QBLK], f16)
            qc8 = rpool.tile([128, DCH * 2 * QBLK], f8)
            # V resident: [128, (kc*KT_SUB + kt) * DV] f32r, loaded once
            v_res = rpool.tile([128, NK // 128 * DV], f32r)
            # load each q-half separately: the first matmuls touch only
            # half 0, so its 256KB slices gate the start, not the full 1MB
            for hq in range(N_QH):
                for c in range(0, DCH, 2):
                    nc.sync.dma_start(
                        qth.rearrange("p (c q) -> p c q", c=DCH)
                           [:, c:c + 2, hq * QH:(hq + 1) * QH],
                        qth_d.rearrange("(c p) q -> p c q", c=DCH)
                             [:, c:c + 2, hq * QH:(hq + 1) * QH])
                if hq == 0:
                    for c in range(0, DCH, 2):
                        nc.scalar.dma_start(
                            qc8[:, c * 2 * QBLK:(c + 2) * 2 * QBLK]
                               .rearrange("p (c f) -> p c f", c=2),
                            qc8_d.rearrange("(c p) f -> p c f", c=DCH)
                                 [:, c:c + 2, :])
            ones = rpool.tile([128, 2], f32r)
            nc.sync.dma_start(ones[:], ones_d[:])
            # warm-up: the ones tile lands ~1.5us in (512B DMA), ~8us before
            # the first real operands; matmuls on it start the PE p-state
            # ramp early so real work begins at full clock
            warm_ps = spsum.tile([2, 2], mybir.dt.float32, tag="s_ps")
            for w in range(8):
                nc.tensor.matmul(warm_ps[:], ones[:], ones[:],
                                 start=(w == 0), stop=(w == 7),
                                 skip_group_check=True)
            bias_c = rpool.tile([128, 1], f32)
            nc.sync.dma_start(bias_c[:], bias_d[:])
            bias_zero = rpool.tile([128, 1], f32)
            nc.gpsimd.memset(bias_zero[:], 0.0)

            qc8_4d = qc8.rearrange("p (c j q) -> p c j q", c=DCH, j=2)

            for qh in range(N_QH):
                o_ps = [opsum.tile([128, DV], f32, name=f"o_ps{qh}_{qt}",
                                   tag=f"o_ps{qt}")
                        for qt in range(QT_PER_H)]
                padd = epool.tile([128, QH], f32, name=f"padd{qh}",
                                  tag="padd", bufs=2)
                padd_r = epool.tile([128, QH], f32r, name=f"padd_r{qh}",
                                    tag="padd_r", bufs=2)

                for kc in range(N_KC):
                    # Stream K^T hi, fp8 cross pack, and V chunks
                    kth_c = spool.tile([128, DCH * KC], f16, tag="kth")
                    kc8_c = spool.tile([128, DCH * 2 * KC], f8, tag="kc8")
                    nc.sync.dma_start(
                        kth_c.rearrange("p (c k) -> p c k", c=DCH),
                        kth_d.rearrange("(c p) k -> p c k", c=DCH)
                             [:, :, kc * KC:(kc + 1) * KC])
                    # kc8 DRAM layout is [D, N_KC, 2, KC] so a chunk's
                    # (j, k) block is contiguous per row (3D-balanceable DMA)
                    nc.sync.dma_start(
                        kc8_c.rearrange("p (c f) -> p c f", c=DCH),
                        kc8_d[:, kc * 2 * KC:(kc + 1) * 2 * KC]
                             .rearrange("(c p) f -> p c f", c=DCH))
                    if qh == 0:
                        nc.sync.dma_start(
                            v_res[:, kc * KT_SUB * DV:(kc + 1) * KT_SUB * DV]
                                 .rearrange("p (s n) -> p s n", s=KT_SUB),
                            v_d[kc * KC:(kc + 1) * KC, :]
                               .rearrange("(s p) n -> p s n", s=KT_SUB))

                    kc8_c4 = kc8_c.rearrange("p (c j k) -> p c j k",
                                             c=DCH, j=2)

                    for kt in range(KT_SUB):
                        # main term: Kh^T @ Qh (fp16, 1 cyc/row)
                        s_ps = spsum.tile([128, QH], f32, name="s_ps")
                        for c in range(DCH):
                            nc.tensor.matmul(
                                s_ps[:],
                                kth_c[:, c * KC + kt * 128:
                                      c * KC + (kt + 1) * 128],
                                qth[:, c * QBLK + qh * QH:
                                    c * QBLK + (qh + 1) * QH],
                                start=(c == 0), stop=(c == DCH - 1),
                                skip_group_check=True)

                        # cross terms: one fp8 DoubleRow matmul per d-chunk
                        c_ps = cpsum.tile([128, QH], f32, name="c_ps")
                        for c in range(DCH):
                            nc.tensor.matmul(
                                c_ps[:],
                                kc8_c4[:, c, :, kt * 128:(kt + 1) * 128],
                                qc8_4d[:, c, :, qh * QH:(qh + 1) * QH],
                                start=(c == 0), stop=(c == DCH - 1),
                                perf_mode=mybir.MatmulPerfMode.DoubleRow,
                                skip_group_check=True)

                        e1 = epool.tile([128, QH], f32, tag="e1")
                        nc.scalar.activation(e1[:], s_ps[:],
                                             mybir.ActivationFunctionType.Exp,
                                             bias=bias_c[:], scale=1.0)
                        e2 = epool.tile([128, QH], f32, tag="e2")
                        nc.scalar.activation(e2[:], c_ps[:],
                                             mybir.ActivationFunctionType.Exp,
                                             bias=bias_zero[:],
                                             scale=1.0 / CROSS_SCALE)
                        pt = ppool.tile([128, QH], f32r, name="pt")
                        nc.vector.tensor_mul(pt[:], e1[:], e2[:])

                        first = kc == 0 and kt == 0
                        last = kc == N_KC - 1 and kt == KT_SUB - 1
                        # running sum of P tiles on the (otherwise idle) DVE;
                        # feeds the 4 end-of-half row-sum matmuls
                        if first:
                            nc.vector.tensor_copy(padd[:], pt[:])
                        elif last:
                            nc.vector.tensor_add(padd_r[:], padd[:], pt[:])
                        else:
                            nc.vector.tensor_add(padd[:], padd[:], pt[:])
                        if last:
                            # row sums first: lets the DVE start the
                            # reciprocal/normalize while PE runs the last PVs.
                            # l shares the s_ps slots; allocating it HERE (not
                            # at half start) keeps the pool rotation sound.
                            l_ps = spsum.tile([128, 2 * QT_PER_H], f32,
                                              name=f"l_ps{qh}", tag="s_ps")
                            for qt in range(QT_PER_H):
                                nc.tensor.matmul(
                                    l_ps[:, 2 * qt:2 * qt + 2],
                                    padd_r[:, qt * 128:(qt + 1) * 128],
                                    ones[:],
                                    start=(qt == 0), stop=(qt == QT_PER_H - 1),
                                    skip_group_check=True)
                        for qt in range(QT_PER_H):
                            nc.tensor.matmul(
                                o_ps[qt][:],
                                pt[:, qt * 128:(qt + 1) * 128],
                                v_res[:, (kc * KT_SUB + kt) * DV:
                                      (kc * KT_SUB + kt + 1) * DV],
                                start=first, stop=last,
                                skip_group_check=True)

                # Normalize: O[q, :] / l[q], store
                for qt in range(QT_PER_H):
                    rcp = opool.tile([128, 1], f32, tag="rcp")
                    nc.vector.reciprocal(rcp[:], l_ps[:, 2 * qt:2 * qt + 1])
                    o_sb = opool.tile([128, DV], f32, tag="o_sb")
                    nc.vector.tensor_scalar_mul(o_sb[:], o_ps[qt][:], rcp[:])
                    nc.sync.dma_start(
                        out_d[qh * QH + qt * 128: qh * QH + (qt + 1) * 128, :],
                        o_sb[:])

    nc.compile()
    return nc


def _get_compiled():
    global _compiled
    if _compiled is None:
        _compiled = _build()
    return _compiled


last_results = None
_last_in_maps = None


def kernel(query: np.ndarray, key: np.ndarray, value: np.ndarray) -> np.ndarray:
    import ml_dtypes
    from concourse import bass_utils

    nc = _get_compiled()

    qt = np.ascontiguousarray(np.asarray(query, dtype=np.float32).T)
    kt = np.ascontiguousarray(np.asarray(key, dtype=np.float32).T)
    qth = qt.astype(np.float16)
    qtl = qt - qth.astype(np.float32)
    kth = kt.astype(np.float16)
    ktl = kt - kth.astype(np.float32)
    v = _round_f32r(np.asarray(value, dtype=np.float32))
    ones = np.ones((128, 2), dtype=np.float32)
    # softmax shift: scores ~ N(0, sigma^2) with sigma = |Q|_rms * |K|_rms
    # * sqrt(D); the max of NK samples sits near 4.2 sigma. Subtracting
    # c ~= that max keeps exp() in range for any input scaling, and a
    # constant shift cancels exactly in the normalization.
    q32 = np.asarray(query, dtype=np.float32)
    k32 = np.asarray(key, dtype=np.float32)
    sigma = (np.sqrt(np.mean(q32 * q32) * np.mean(k32 * k32) * D))
    c_shift = float(4.3 * sigma)
    bias = np.full((128, 1), -c_shift, dtype=np.float32)

    f8 = ml_dtypes.float8_e4m3
    # fp8 cross packs: K side [d, kc, (j k)] with j=0: 2^11*Kl, j=1: Kh;
    # Q side [d, (j q)] with j=0: Qh, j=1: 2^11*Ql
    kc8 = np.empty((D, N_KC, 2, KC), dtype=f8)
    kc8[:, :, 0, :] = (ktl * CROSS_SCALE).astype(f8).reshape(D, N_KC, KC)
    kc8[:, :, 1, :] = kth.astype(np.float32).astype(f8).reshape(D, N_KC, KC)
    kc8 = kc8.reshape(D, 2 * NK)
    qc8_full = np.empty((D, 2, NQ), dtype=f8)
    qc8_full[:, 0, :] = qth.astype(np.float32).astype(f8)
    qc8_full[:, 1, :] = (qtl * CROSS_SCALE).astype(f8)

    in_maps = []
    for c in range(N_CORES):
        in_maps.append({
            "qth": np.ascontiguousarray(qth[:, c * QBLK:(c + 1) * QBLK]),
            "qc8": np.ascontiguousarray(
                qc8_full[:, :, c * QBLK:(c + 1) * QBLK]).reshape(D, 2 * QBLK),
            "kth": kth,
            "kc8": kc8,
            "v": v,
            "ones": ones,
            "bias": bias,
        })

    res = bass_utils.run_bass_kernel_spmd(nc, in_maps,
                                          core_ids=list(range(N_CORES)))
    global last_results, _last_in_maps
    last_results = res
    _last_in_maps = in_maps
    return np.concatenate([r["out"] for r in res.results], axis=0)



# revision 2
# speedup vs baseline: 1.2431x; 1.2431x over previous
"""Cross-attention kernel for Trainium2, sharded across 8 NeuronCores.

out = softmax(Q @ K^T) @ V with Q,K: [8192,512], V: [8192,512], fp32.

Sharding: query rows across the 8 cores (1024 rows each); K/V replicated.

Per-core algorithm (all in the S^T = K@Q^T layout so that no on-chip
transposes are needed):
  - Host pre-transposes Q and K and rounds to fp16 (11-bit mantissa).
    S^T = Kh^T @ Qh as an fp16 matmul (1 cycle/row on the PE vs 4 for
    fp32). The fp16 rounding of Q and K puts ~7e-3 std error on the
    scores, i.e. ~1.5e-3 relative output error -- an order of magnitude
    inside the 2e-2 gate, and it halves both the PE work and the K-side
    DMA vs an fp32 pipeline.
  - exp(S - c): one ACT activation per S tile, writing float32r so the
    P@V matmul runs at 1 cycle/row. The constant bias -c replaces the
    row max: scores are N(0, 512), so row maxes concentrate in [80, 115];
    exp(S-c) neither overflows nor flushes an entire row to zero, and a
    constant shift cancels exactly in the normalization.
  - row sums (softmax denominators): P tiles are accumulated on the
    (otherwise idle) DVE, then reduced over partitions by tiny N=2
    matmuls against a ones vector at the end of each q-half.
  - P@V accumulates over all of K in PSUM, q-half at a time. PSUM banks:
    4 O + 1 rowsum + 2 S^T = 7.
"""

import numpy as np

N_CORES = 8
NQ, NK, D, DV = 8192, 8192, 512, 512
QBLK = NQ // N_CORES          # 1024 query rows per core
QH = 512                      # q-half (moving-operand width for S^T matmul)
N_QH = QBLK // QH             # 2
KC = 512                      # k-chunk rows streamed per DMA
N_KC = NK // KC               # 16
KT_SUB = KC // 128            # 4 k-subtiles per chunk
DCH = D // 128                # 4 contraction chunks
QT_PER_H = QH // 128          # 4 q-tiles per half

_compiled = None


def _round_f32r(x: np.ndarray) -> np.ndarray:
    """Round fp32 to f32r (11-bit mantissa, RTNE), matching the HW rounding."""
    b = np.ascontiguousarray(x).view(np.uint32)
    r = ((b >> np.uint32(12)) & np.uint32(1)) + np.uint32(0x7FF)
    return ((b + r) & np.uint32(0xFFFFF000)).view(np.float32)


def _build():
    import concourse.mybir as mybir
    import concourse.tile as tile
    from concourse import bacc

    f32 = mybir.dt.float32
    f32r = mybir.dt.float32r
    f16 = mybir.dt.float16

    nc = bacc.Bacc("TRN2", target_bir_lowering=False, debug=False,
                   num_devices=N_CORES)

    qth_d = nc.dram_tensor("qth", [D, QBLK], f16, kind="ExternalInput").ap()
    kth_d = nc.dram_tensor("kth", [D, NK], f16, kind="ExternalInput").ap()
    v_d = nc.dram_tensor("v", [NK, DV], f32r, kind="ExternalInput").ap()
    ones_d = nc.dram_tensor("ones", [128, 2], f32r, kind="ExternalInput").ap()
    bias_d = nc.dram_tensor("bias", [128, 1], f32, kind="ExternalInput").ap()
    out_d = nc.dram_tensor("out", [QBLK, DV], f32, kind="ExternalOutput").ap()

    with tile.TileContext(nc) as tc:
        with tc.tile_pool(name="resident", bufs=1) as rpool, \
             tc.tile_pool(name="stream", bufs=3) as spool, \
             tc.tile_pool(name="ptile", bufs=4) as ppool, \
             tc.tile_pool(name="padds", bufs=2) as apool, \
             tc.tile_pool(name="outp", bufs=4) as opool, \
             tc.tile_pool(name="spsum", bufs=2, space="PSUM") as spsum, \
             tc.tile_pool(name="opsum", bufs=1, space="PSUM") as opsum:

            # Resident: Q^T hi as [128, DCH, QBLK]
            qth = rpool.tile([128, DCH * QBLK], f16)
            # V resident: [128, (kc*KT_SUB + kt) * DV] f32r, loaded once
            v_res = rpool.tile([128, NK // 128 * DV], f32r)
            # load each q-half separately: the first matmuls touch only
            # half 0, so its slices gate the start, not the full tensor
            for hq in range(N_QH):
                for c in range(0, DCH, 2):
                    nc.sync.dma_start(
                        qth.rearrange("p (c q) -> p c q", c=DCH)
                           [:, c:c + 2, hq * QH:(hq + 1) * QH],
                        qth_d.rearrange("(c p) q -> p c q", c=DCH)
                             [:, c:c + 2, hq * QH:(hq + 1) * QH])
            ones = rpool.tile([128, 2], f32r)
            nc.sync.dma_start(ones[:], ones_d[:])
            # warm-up: the ones tile lands ~1.5us in (512B DMA), ~8us before
            # the first real operands; matmuls on it start the PE p-state
            # ramp early so real work begins at full clock
            warm_ps = spsum.tile([2, 2], mybir.dt.float32, tag="s_ps")
            for w in range(8):
                nc.tensor.matmul(warm_ps[:], ones[:], ones[:],
                                 start=(w == 0), stop=(w == 7),
                                 skip_group_check=True)
            bias_c = rpool.tile([128, 1], f32)
            nc.sync.dma_start(bias_c[:], bias_d[:])

            for qh in range(N_QH):
                o_ps = [opsum.tile([128, DV], f32, name=f"o_ps{qh}_{qt}",
                                   tag=f"o_ps{qt}")
                        for qt in range(QT_PER_H)]
                padd = apool.tile([128, QH], f32, name=f"padd{qh}",
                                  tag="padd", bufs=2)
                padd_r = apool.tile([128, QH], f32r, name=f"padd_r{qh}",
                                    tag="padd_r", bufs=2)

                for kc in range(N_KC):
                    # Stream K^T hi and V chunks
                    kth_c = spool.tile([128, DCH * KC], f16, tag="kth")
                    nc.sync.dma_start(
                        kth_c.rearrange("p (c k) -> p c k", c=DCH),
                        kth_d.rearrange("(c p) k -> p c k", c=DCH)
                             [:, :, kc * KC:(kc + 1) * KC])
                    if qh == 0:
                        nc.sync.dma_start(
                            v_res[:, kc * KT_SUB * DV:(kc + 1) * KT_SUB * DV]
                                 .rearrange("p (s n) -> p s n", s=KT_SUB),
                            v_d[kc * KC:(kc + 1) * KC, :]
                               .rearrange("(s p) n -> p s n", s=KT_SUB))

                    for kt in range(KT_SUB):
                        # S^T tile: Kh^T @ Qh (fp16, 1 cyc/row)
                        s_ps = spsum.tile([128, QH], f32, name="s_ps")
                        for c in range(DCH):
                            nc.tensor.matmul(
                                s_ps[:],
                                kth_c[:, c * KC + kt * 128:
                                      c * KC + (kt + 1) * 128],
                                qth[:, c * QBLK + qh * QH:
                                    c * QBLK + (qh + 1) * QH],
                                start=(c == 0), stop=(c == DCH - 1),
                                skip_group_check=True)

                        # P = exp(S - c), written as f32r so the P@V matmul
                        # runs at 1 cycle/row
                        pt = ppool.tile([128, QH], f32r, name="pt")
                        nc.scalar.activation(pt[:], s_ps[:],
                                             mybir.ActivationFunctionType.Exp,
                                             bias=bias_c[:], scale=1.0)

                        first = kc == 0 and kt == 0
                        last = kc == N_KC - 1 and kt == KT_SUB - 1
                        # running sum of P tiles on the (otherwise idle) DVE;
                        # feeds the 4 end-of-half row-sum matmuls
                        if first:
                            nc.vector.tensor_copy(padd[:], pt[:])
                        elif last:
                            nc.vector.tensor_add(padd_r[:], padd[:], pt[:])
                        else:
                            nc.vector.tensor_add(padd[:], padd[:], pt[:])
                        if last:
                            # row sums first: lets the DVE start the
                            # reciprocal/normalize while PE runs the last PVs.
                            # l shares the s_ps slots; allocating it HERE (not
                            # at half start) keeps the pool rotation sound.
                            l_ps = spsum.tile([128, 2 * QT_PER_H], f32,
                                              name=f"l_ps{qh}", tag="s_ps")
                            for qt in range(QT_PER_H):
                                nc.tensor.matmul(
                                    l_ps[:, 2 * qt:2 * qt + 2],
                                    padd_r[:, qt * 128:(qt + 1) * 128],
                                    ones[:],
                                    start=(qt == 0), stop=(qt == QT_PER_H - 1),
                                    skip_group_check=True)
                        for qt in range(QT_PER_H):
                            nc.tensor.matmul(
                                o_ps[qt][:],
                                pt[:, qt * 128:(qt + 1) * 128],
                                v_res[:, (kc * KT_SUB + kt) * DV:
                                      (kc * KT_SUB + kt + 1) * DV],
                                start=first, stop=last,
                                skip_group_check=True)

                # Normalize: O[q, :] / l[q], store
                for qt in range(QT_PER_H):
                    rcp = opool.tile([128, 1], f32, tag="rcp")
                    nc.vector.reciprocal(rcp[:], l_ps[:, 2 * qt:2 * qt + 1])
                    o_sb = opool.tile([128, DV], f32, tag="o_sb")
                    nc.vector.tensor_scalar_mul(o_sb[:], o_ps[qt][:], rcp[:])
                    nc.sync.dma_start(
                        out_d[qh * QH + qt * 128: qh * QH + (qt + 1) * 128, :],
                        o_sb[:])

    nc.compile()
    return nc


def _get_compiled():
    global _compiled
    if _compiled is None:
        _compiled = _build()
    return _compiled


last_results = None
_last_in_maps = None


def kernel(query: np.ndarray, key: np.ndarray, value: np.ndarray) -> np.ndarray:
    from concourse import bass_utils

    nc = _get_compiled()

    qth = np.ascontiguousarray(np.asarray(query, dtype=np.float32).T
                               ).astype(np.float16)
    kth = np.ascontiguousarray(np.asarray(key, dtype=np.float32).T
                               ).astype(np.float16)
    v = _round_f32r(np.asarray(value, dtype=np.float32))
    ones = np.ones((128, 2), dtype=np.float32)
    # softmax shift: scores ~ N(0, sigma^2) with sigma = |Q|_rms * |K|_rms
    # * sqrt(D); the max of NK samples sits near 4.2 sigma. Subtracting
    # c ~= that max keeps exp() in range for any input scaling, and a
    # constant shift cancels exactly in the normalization.
    q32 = np.asarray(query, dtype=np.float32)
    k32 = np.asarray(key, dtype=np.float32)
    sigma = (np.sqrt(np.mean(q32 * q32) * np.mean(k32 * k32) * D))
    c_shift = float(4.3 * sigma)
    bias = np.full((128, 1), -c_shift, dtype=np.float32)

    in_maps = []
    for c in range(N_CORES):
        in_maps.append({
            "qth": np.ascontiguousarray(qth[:, c * QBLK:(c + 1) * QBLK]),
            "kth": kth,
            "v": v,
            "ones": ones,
            "bias": bias,
        })

    res = bass_utils.run_bass_kernel_spmd(nc, in_maps,
                                          core_ids=list(range(N_CORES)))
    global last_results, _last_in_maps
    last_results = res
    _last_in_maps = in_maps
    return np.concatenate([r["out"] for r in res.results], axis=0)


# revision 5
# speedup vs baseline: 1.2944x; 1.0412x over previous
"""Cross-attention kernel for Trainium2, sharded across 8 NeuronCores.

out = softmax(Q @ K^T) @ V with Q,K: [8192,512], V: [8192,512], fp32.

Sharding: query rows across the 8 cores (1024 rows each); K/V replicated.

Per-core algorithm (all in the S^T = K@Q^T layout so that no on-chip
transposes are needed):
  - Host pre-transposes Q and K and rounds to fp16 (11-bit mantissa).
    S^T = Kh^T @ Qh as an fp16 matmul (1 cycle/row on the PE vs 4 for
    fp32). The fp16 rounding of Q and K puts ~7e-3 std error on the
    scores, i.e. ~1.5e-3 relative output error -- an order of magnitude
    inside the 2e-2 gate, and it halves both the PE work and the K-side
    DMA vs an fp32 pipeline.
  - exp(S - c): one ACT activation per S tile, writing float32r so the
    P@V matmul runs at 1 cycle/row. The constant bias -c replaces the
    row max: scores are N(0, 512), so row maxes concentrate in [80, 115];
    exp(S-c) neither overflows nor flushes an entire row to zero, and a
    constant shift cancels exactly in the normalization.
  - P@V is software-pipelined one k-tile behind S^T: the PE runs
    S(kt) then PV(kt-1), so the exp(kt) ACT latency hides under S(kt+1)
    and the PE never stalls on the activation chain.
  - row sums (softmax denominators): P tiles are accumulated on the
    (otherwise idle) DVE, then reduced over partitions by tiny N=2
    matmuls against a ones vector after the last PV.
  - normalization alternates ACT (activation Copy with a per-partition
    reciprocal scale) and DVE so the four output tiles drain in two
    rounds instead of four.
  - PSUM banks: 2 S^T (+rowsum, shared) + 4 O of the current half + 2 O
    of the other half (double-buffered so the next half's PV does not
    wait on this half's normalize) = 8.
  - PE p-state: the cost model ramps 0.65 -> 1.2 -> 2.4 GHz over 3us of
    continuous PE busy; a run of matmuls on a memset tile (no DMA
    dependency) spans the head DMA so real work starts at full clock.
"""

import numpy as np

N_CORES = 8
NQ, NK, D, DV = 8192, 8192, 512, 512
QBLK = NQ // N_CORES          # 1024 query rows per core
QH = 512                      # q-half (moving-operand width for S^T matmul)
N_QH = QBLK // QH             # 2
KC = 512                      # k-chunk rows streamed per DMA
N_KC = NK // KC               # 16
KT_SUB = KC // 128            # 4 k-subtiles per chunk
DCH = D // 128                # 4 contraction chunks
QT_PER_H = QH // 128          # 4 q-tiles per half
N_WARM = 13                   # p-state warm-up matmuls (free=512 each)

_compiled = None


def _round_f32r(x: np.ndarray) -> np.ndarray:
    """Round fp32 to f32r (11-bit mantissa, RTNE), matching the HW rounding."""
    b = np.ascontiguousarray(x).view(np.uint32)
    r = ((b >> np.uint32(12)) & np.uint32(1)) + np.uint32(0x7FF)
    return ((b + r) & np.uint32(0xFFFFF000)).view(np.float32)


def _build():
    import concourse.mybir as mybir
    import concourse.tile as tile
    from concourse import bacc

    f32 = mybir.dt.float32
    f32r = mybir.dt.float32r
    f16 = mybir.dt.float16

    nc = bacc.Bacc("TRN2", target_bir_lowering=False, debug=False,
                   num_devices=N_CORES)

    qth_d = nc.dram_tensor("qth", [D, QBLK], f16, kind="ExternalInput").ap()
    kth_d = nc.dram_tensor("kth", [D, NK], f16, kind="ExternalInput").ap()
    v_d = nc.dram_tensor("v", [NK, DV], f32r, kind="ExternalInput").ap()
    ones_d = nc.dram_tensor("ones", [128, 2], f32r, kind="ExternalInput").ap()
    bias_d = nc.dram_tensor("bias", [128, 1], f32, kind="ExternalInput").ap()
    out_d = nc.dram_tensor("out", [QBLK, DV], f32, kind="ExternalOutput").ap()

    with tile.TileContext(nc) as tc:
        with tc.tile_pool(name="resident", bufs=1) as rpool, \
             tc.tile_pool(name="stream", bufs=3) as spool, \
             tc.tile_pool(name="ptile", bufs=4) as ppool, \
             tc.tile_pool(name="padds", bufs=2) as apool, \
             tc.tile_pool(name="outp", bufs=4) as opool, \
             tc.tile_pool(name="spsum", bufs=2, space="PSUM") as spsum, \
             tc.tile_pool(name="opsum", bufs=1, space="PSUM") as opsum:

            # p-state warm-up: memset tile (no DMA dependency) keeps the PE
            # busy from ~0.7us until the first K/Q chunks land, so the ramp
            # (full clock after 3us of continuous busy) completes before any
            # real matmul issues.
            wz = rpool.tile([128, QH], f32)
            nc.gpsimd.memset(wz[:], 0.0)
            warm_ps = spsum.tile([128, QH], f32, tag="s_ps")
            for w in range(N_WARM):
                nc.tensor.matmul(warm_ps[:], wz[:, :128].bitcast(f32r),
                                 wz[:].bitcast(f32r),
                                 start=(w == 0), stop=(w == N_WARM - 1),
                                 skip_group_check=True)

            # Resident: Q^T hi as [128, DCH, QBLK]
            qth = rpool.tile([128, DCH * QBLK], f16)
            # V resident: [128, (kc*KT_SUB + kt) * DV] f32r, loaded once
            v_res = rpool.tile([128, NK // 128 * DV], f32r)
            # Head-critical loads on the sync (SP) queue: q-half 0 then the
            # kc=0 K chunk gate the first S matmuls; V kc=0 follows (first
            # needed one exp-latency later).
            nc.sync.dma_start(
                qth.rearrange("p (c q) -> p c q", c=DCH)[:, 0:2, 0:QH],
                qth_d.rearrange("(c p) q -> p c q", c=DCH)[:, 0:2, 0:QH])
            nc.sync.dma_start(
                qth.rearrange("p (c q) -> p c q", c=DCH)[:, 2:4, 0:QH],
                qth_d.rearrange("(c p) q -> p c q", c=DCH)[:, 2:4, 0:QH])
            # Non-critical small/late loads on the scalar (ACT) queue.
            bias_c = rpool.tile([128, 1], f32)
            nc.scalar.dma_start(bias_c[:], bias_d[:])
            ones = rpool.tile([128, 2], f32r)
            nc.scalar.dma_start(ones[:], ones_d[:])
            for c in range(0, DCH, 2):
                nc.scalar.dma_start(
                    qth.rearrange("p (c q) -> p c q", c=DCH)
                       [:, c:c + 2, QH:2 * QH],
                    qth_d.rearrange("(c p) q -> p c q", c=DCH)
                         [:, c:c + 2, QH:2 * QH])

            for qh in range(N_QH):
                # qt0/qt1 PSUM banks alternate between halves so the next
                # half's first PVs don't wait on this half's normalize.
                o_ps = [opsum.tile([128, DV], f32, name=f"o_ps{qh}_{qt}",
                                   tag=(f"o_ps{qt}_{qh % 2}" if qt < 2
                                        else f"o_ps{qt}"))
                        for qt in range(QT_PER_H)]
                padd = apool.tile([128, QH], f32, name=f"padd{qh}",
                                  tag="padd", bufs=2)
                padd_r = apool.tile([128, QH], f32r, name=f"padd_r{qh}",
                                    tag="padd_r", bufs=2)

                pend = None   # (pt tile, k-tile index, is-first) awaiting PV
                for kc in range(N_KC):
                    # Stream K^T hi and V chunks
                    kth_c = spool.tile([128, DCH * KC], f16, tag="kth")
                    nc.sync.dma_start(
                        kth_c.rearrange("p (c k) -> p c k", c=DCH),
                        kth_d.rearrange("(c p) k -> p c k", c=DCH)
                             [:, :, kc * KC:(kc + 1) * KC])
                    if qh == 0:
                        nc.sync.dma_start(
                            v_res[:, kc * KT_SUB * DV:(kc + 1) * KT_SUB * DV]
                                 .rearrange("p (s n) -> p s n", s=KT_SUB),
                            v_d[kc * KC:(kc + 1) * KC, :]
                               .rearrange("(s p) n -> p s n", s=KT_SUB))

                    for kt in range(KT_SUB):
                        # S^T tile: Kh^T @ Qh (fp16, 1 cyc/row)
                        s_ps = spsum.tile([128, QH], f32, name="s_ps")
                        for c in range(DCH):
                            nc.tensor.matmul(
                                s_ps[:],
                                kth_c[:, c * KC + kt * 128:
                                      c * KC + (kt + 1) * 128],
                                qth[:, c * QBLK + qh * QH:
                                    c * QBLK + (qh + 1) * QH],
                                start=(c == 0), stop=(c == DCH - 1),
                                skip_group_check=True)

                        # P = exp(S - c), written as f32r so the P@V matmul
                        # runs at 1 cycle/row
                        pt = ppool.tile([128, QH], f32r, name="pt")
                        nc.scalar.activation(pt[:], s_ps[:],
                                             mybir.ActivationFunctionType.Exp,
                                             bias=bias_c[:], scale=1.0)

                        first = kc == 0 and kt == 0
                        last = kc == N_KC - 1 and kt == KT_SUB - 1
                        # running sum of P tiles on the (otherwise idle) DVE;
                        # feeds the 4 end-of-half row-sum matmuls
                        if first:
                            nc.vector.tensor_copy(padd[:], pt[:])
                        elif last:
                            nc.vector.tensor_add(padd_r[:], padd[:], pt[:])
                        else:
                            nc.vector.tensor_add(padd[:], padd[:], pt[:])

                        # PV for the PREVIOUS k-tile: its pt has been ready
                        # for a full iteration, so the PE goes straight from
                        # S(kt) into PV(kt-1) with no activation-chain stall.
                        if pend is not None:
                            ptp, kp, firstp = pend
                            for qt in range(QT_PER_H):
                                nc.tensor.matmul(
                                    o_ps[qt][:],
                                    ptp[:, qt * 128:(qt + 1) * 128],
                                    v_res[:, kp * DV:(kp + 1) * DV],
                                    start=firstp, stop=False,
                                    skip_group_check=True)
                        pend = (pt, kc * KT_SUB + kt, first)

                # drain: PV for the final k-tile closes the O accumulation
                ptp, kp, firstp = pend
                for qt in range(QT_PER_H):
                    nc.tensor.matmul(
                        o_ps[qt][:],
                        ptp[:, qt * 128:(qt + 1) * 128],
                        v_res[:, kp * DV:(kp + 1) * DV],
                        start=firstp, stop=True,
                        skip_group_check=True)

                # row sums: reduce padd_r over partitions with tiny matmuls
                # against ones; lands right as the last PVs finish.
                l_ps = spsum.tile([128, 2 * QT_PER_H], f32,
                                  name=f"l_ps{qh}", tag="s_ps")
                for qt in range(QT_PER_H):
                    nc.tensor.matmul(
                        l_ps[:, 2 * qt:2 * qt + 2],
                        padd_r[:, qt * 128:(qt + 1) * 128],
                        ones[:],
                        start=(qt == 0), stop=(qt == QT_PER_H - 1),
                        skip_group_check=True)

                # Normalize O[q, :] / l[q] and store; reciprocals on DVE,
                # multiplies alternating ACT/DVE so two tiles drain at once.
                rcps = []
                for qt in range(QT_PER_H):
                    rcp = opool.tile([128, 1], f32, tag="rcp")
                    nc.vector.reciprocal(rcp[:], l_ps[:, 2 * qt:2 * qt + 1])
                    rcps.append(rcp)
                for qt in range(QT_PER_H):
                    o_sb = opool.tile([128, DV], f32, tag="o_sb")
                    nc.vector.tensor_scalar_mul(o_sb[:], o_ps[qt][:],
                                                rcps[qt][:])
                    nc.sync.dma_start(
                        out_d[qh * QH + qt * 128: qh * QH + (qt + 1) * 128, :],
                        o_sb[:])

    nc.compile()
    return nc


def _get_compiled():
    global _compiled
    if _compiled is None:
        _compiled = _build()
    return _compiled


last_results = None
_last_in_maps = None


def kernel(query: np.ndarray, key: np.ndarray, value: np.ndarray) -> np.ndarray:
    from concourse import bass_utils

    nc = _get_compiled()

    qth = np.ascontiguousarray(np.asarray(query, dtype=np.float32).T
                               ).astype(np.float16)
    kth = np.ascontiguousarray(np.asarray(key, dtype=np.float32).T
                               ).astype(np.float16)
    v = _round_f32r(np.asarray(value, dtype=np.float32))
    ones = np.ones((128, 2), dtype=np.float32)
    # softmax shift: scores ~ N(0, sigma^2) with sigma = |Q|_rms * |K|_rms
    # * sqrt(D); the max of NK samples sits near 4.2 sigma. Subtracting
    # c ~= that max keeps exp() in range for any input scaling, and a
    # constant shift cancels exactly in the normalization.
    q32 = np.asarray(query, dtype=np.float32)
    k32 = np.asarray(key, dtype=np.float32)
    sigma = (np.sqrt(np.mean(q32 * q32) * np.mean(k32 * k32) * D))
    c_shift = float(4.3 * sigma)
    bias = np.full((128, 1), -c_shift, dtype=np.float32)

    in_maps = []
    for c in range(N_CORES):
        in_maps.append({
            "qth": np.ascontiguousarray(qth[:, c * QBLK:(c + 1) * QBLK]),
            "kth": kth,
            "v": v,
            "ones": ones,
            "bias": bias,
        })

    res = bass_utils.run_bass_kernel_spmd(nc, in_maps,
                                          core_ids=list(range(N_CORES)))
    global last_results, _last_in_maps
    last_results = res
    _last_in_maps = in_maps
    return np.concatenate([r["out"] for r in res.results], axis=0)


# revision 9
# speedup vs baseline: 1.2972x; 1.0021x over previous
"""Cross-attention kernel for Trainium2, sharded across 8 NeuronCores.

out = softmax(Q @ K^T) @ V with Q,K: [8192,512], V: [8192,512], fp32.

Sharding: query rows across the 8 cores (1024 rows each); K/V replicated.

Per-core algorithm (all in the S^T = K@Q^T layout so that no on-chip
transposes are needed):
  - Host pre-transposes Q and K and rounds to fp16 (11-bit mantissa).
    S^T = Kh^T @ Qh as an fp16 matmul (1 cycle/row on the PE vs 4 for
    fp32). The fp16 rounding of Q and K puts ~7e-3 std error on the
    scores, i.e. ~1.5e-3 relative output error -- an order of magnitude
    inside the 2e-2 gate, and it halves both the PE work and the K-side
    DMA vs an fp32 pipeline.
  - exp(S - c): one ACT activation per S tile, writing float32r so the
    P@V matmul runs at 1 cycle/row. The constant bias -c replaces the
    row max: scores are N(0, 512), so row maxes concentrate in [80, 115];
    exp(S-c) neither overflows nor flushes an entire row to zero, and a
    constant shift cancels exactly in the normalization.
  - P@V is software-pipelined one k-tile behind S^T: the PE runs
    S(kt) then PV(kt-1), so the exp(kt) ACT latency hides under S(kt+1)
    and the PE never stalls on the activation chain.
  - row sums (softmax denominators): P tiles are accumulated on the
    (otherwise idle) DVE, then reduced over partitions by tiny N=2
    matmuls against a ones vector after the last PV.
  - normalization alternates ACT (activation Copy with a per-partition
    reciprocal scale) and DVE so the four output tiles drain in two
    rounds instead of four.
  - PSUM banks: 2 S^T (+rowsum, shared) + 4 O of the current half + 2 O
    of the other half (double-buffered so the next half's PV does not
    wait on this half's normalize) = 8.
  - PE p-state: the cost model ramps 0.65 -> 1.2 -> 2.4 GHz over 3us of
    continuous PE busy; a run of matmuls on a memset tile (no DMA
    dependency) spans the head DMA so real work starts at full clock.
"""

import numpy as np

N_CORES = 8
NQ, NK, D, DV = 8192, 8192, 512, 512
QBLK = NQ // N_CORES          # 1024 query rows per core
QH = 512                      # q-half (moving-operand width for S^T matmul)
N_QH = QBLK // QH             # 2
KC = 512                      # k-chunk rows streamed per DMA
N_KC = NK // KC               # 16
KT_SUB = KC // 128            # 4 k-subtiles per chunk
DCH = D // 128                # 4 contraction chunks
QT_PER_H = QH // 128          # 4 q-tiles per half
N_WARM = 30                   # p-state warm-up matmuls (free=256 each)

_compiled = None


def _round_f32r(x: np.ndarray) -> np.ndarray:
    """Round fp32 to f32r (11-bit mantissa, RTNE), matching the HW rounding."""
    b = np.ascontiguousarray(x).view(np.uint32)
    r = ((b >> np.uint32(12)) & np.uint32(1)) + np.uint32(0x7FF)
    return ((b + r) & np.uint32(0xFFFFF000)).view(np.float32)


def _build():
    import concourse.mybir as mybir
    import concourse.tile as tile
    from concourse import bacc

    f32 = mybir.dt.float32
    f32r = mybir.dt.float32r
    f16 = mybir.dt.float16

    nc = bacc.Bacc("TRN2", target_bir_lowering=False, debug=False,
                   num_devices=N_CORES)

    qth_d = nc.dram_tensor("qth", [D, QBLK], f16, kind="ExternalInput").ap()
    kth_d = nc.dram_tensor("kth", [D, NK], f16, kind="ExternalInput").ap()
    v_d = nc.dram_tensor("v", [NK, DV], f32r, kind="ExternalInput").ap()
    ones_d = nc.dram_tensor("ones", [128, 2], f32r, kind="ExternalInput").ap()
    bias_d = nc.dram_tensor("bias", [128, 1], f32, kind="ExternalInput").ap()
    out_d = nc.dram_tensor("out", [QBLK, DV], f32, kind="ExternalOutput").ap()

    with tile.TileContext(nc) as tc:
        with tc.tile_pool(name="resident", bufs=1) as rpool, \
             tc.tile_pool(name="stream", bufs=3) as spool, \
             tc.tile_pool(name="ptile", bufs=4) as ppool, \
             tc.tile_pool(name="padds", bufs=2) as apool, \
             tc.tile_pool(name="outp", bufs=4) as opool, \
             tc.tile_pool(name="spsum", bufs=2, space="PSUM") as spsum, \
             tc.tile_pool(name="opsum", bufs=1, space="PSUM") as opsum:

            # p-state warm-up: memset tile (no DMA dependency) keeps the PE
            # busy from ~0.7us until the first K/Q chunks land, so the ramp
            # (full clock after 3us of continuous busy) completes before any
            # real matmul issues.
            wz = rpool.tile([128, QH], f32)
            nc.gpsimd.memset(wz[:], 0.0)
            warm_ps = spsum.tile([128, QH], f32, tag="s_ps")
            for w in range(N_WARM):
                nc.tensor.matmul(warm_ps[:, :256],
                                 wz[:, :128].bitcast(f32r),
                                 wz[:, :256].bitcast(f32r),
                                 start=(w == 0), stop=(w == N_WARM - 1),
                                 skip_group_check=True)

            # Resident: Q^T hi as [128, DCH, QBLK]
            qth = rpool.tile([128, DCH * QBLK], f16)
            # V resident: [128, (kc*KT_SUB + kt) * DV] f32r, loaded once
            v_res = rpool.tile([128, NK // 128 * DV], f32r)
            # Head-critical loads on the sync (SP) queue: q-half 0 then the
            # kc=0 K chunk gate the first S matmuls; V kc=0 follows in
            # kt-sized pieces so PV(kt) unblocks as early as possible.
            nc.sync.dma_start(
                qth.rearrange("p (c q) -> p c q", c=DCH)[:, 0:2, 0:QH],
                qth_d.rearrange("(c p) q -> p c q", c=DCH)[:, 0:2, 0:QH])
            nc.sync.dma_start(
                qth.rearrange("p (c q) -> p c q", c=DCH)[:, 2:4, 0:QH],
                qth_d.rearrange("(c p) q -> p c q", c=DCH)[:, 2:4, 0:QH])
            kth_c0 = spool.tile([128, DCH * KC], f16, tag="kth", name="kth_c0")
            nc.sync.dma_start(
                kth_c0.rearrange("p (c k) -> p c k", c=DCH),
                kth_d.rearrange("(c p) k -> p c k", c=DCH)[:, :, 0:KC])
            for b in range(KT_SUB):
                nc.sync.dma_start(
                    v_res[:, b * DV:(b + 1) * DV],
                    v_d[b * 128:(b + 1) * 128, :])
            # Small constants on the gpsimd (SWDGE) queue: no HWDGE slot
            # stolen from the head-critical loads above.
            bias_c = rpool.tile([128, 1], f32)
            nc.gpsimd.dma_start(bias_c[:], bias_d[:])
            ones = rpool.tile([128, 2], f32r)
            nc.gpsimd.dma_start(ones[:], ones_d[:])

            # K^T chunk stream across both halves, prefetched one chunk
            # ahead so neither the kc nor the qh boundary stalls the PE.
            kth_tiles = {0: kth_c0}

            def issue_kth(i):
                if i >= N_QH * N_KC or i in kth_tiles:
                    return
                kc_ = i % N_KC
                t = spool.tile([128, DCH * KC], f16, tag="kth",
                               name=f"kth_{i}")
                nc.sync.dma_start(
                    t.rearrange("p (c k) -> p c k", c=DCH),
                    kth_d.rearrange("(c p) k -> p c k", c=DCH)
                         [:, :, kc_ * KC:(kc_ + 1) * KC])
                kth_tiles[i] = t

            for qh in range(N_QH):
                # qt0/qt1 PSUM banks alternate between halves so the next
                # half's first PVs don't wait on this half's normalize.
                o_ps = [opsum.tile([128, DV], f32, name=f"o_ps{qh}_{qt}",
                                   tag=(f"o_ps{qt}_{qh % 2}" if qt < 2
                                        else f"o_ps{qt}"))
                        for qt in range(QT_PER_H)]
                padd = apool.tile([128, QH], f32, name=f"padd{qh}",
                                  tag="padd", bufs=2)
                padd_r = apool.tile([128, QH], f32r, name=f"padd_r{qh}",
                                    tag="padd_r", bufs=2)

                pend = None   # (pt tile, k-tile index, is-first) awaiting PV
                for kc in range(N_KC):
                    issue_kth(qh * N_KC + kc)
                    if qh == 0 and kc >= 1:
                        # stream the rest of V (kc=0 went with the head)
                        nc.sync.dma_start(
                            v_res[:, kc * KT_SUB * DV:(kc + 1) * KT_SUB * DV]
                                 .rearrange("p (s n) -> p s n", s=KT_SUB),
                            v_d[kc * KC:(kc + 1) * KC, :]
                               .rearrange("(s p) n -> p s n", s=KT_SUB))
                    issue_kth(qh * N_KC + kc + 1)
                    if qh == 0 and kc == 2:
                        # q-half 1, needed in ~110us: issued here so its
                        # HWDGE slots sit behind all head-critical loads
                        for c in range(0, DCH, 2):
                            nc.scalar.dma_start(
                                qth.rearrange("p (c q) -> p c q", c=DCH)
                                   [:, c:c + 2, QH:2 * QH],
                                qth_d.rearrange("(c p) q -> p c q", c=DCH)
                                     [:, c:c + 2, QH:2 * QH])
                    kth_c = kth_tiles.pop(qh * N_KC + kc)

                    for kt in range(KT_SUB):
                        # S^T tile: Kh^T @ Qh (fp16, 1 cyc/row)
                        s_ps = spsum.tile([128, QH], f32, name="s_ps")
                        for c in range(DCH):
                            nc.tensor.matmul(
                                s_ps[:],
                                kth_c[:, c * KC + kt * 128:
                                      c * KC + (kt + 1) * 128],
                                qth[:, c * QBLK + qh * QH:
                                    c * QBLK + (qh + 1) * QH],
                                start=(c == 0), stop=(c == DCH - 1),
                                skip_group_check=True)

                        # P = exp(S - c), written as f32r so the P@V matmul
                        # runs at 1 cycle/row
                        pt = ppool.tile([128, QH], f32r, name="pt")
                        nc.scalar.activation(pt[:], s_ps[:],
                                             mybir.ActivationFunctionType.Exp,
                                             bias=bias_c[:], scale=1.0)

                        first = kc == 0 and kt == 0
                        last = kc == N_KC - 1 and kt == KT_SUB - 1
                        # running sum of P tiles on the (otherwise idle) DVE;
                        # feeds the 4 end-of-half row-sum matmuls
                        if first:
                            nc.vector.tensor_copy(padd[:], pt[:])
                        elif last:
                            nc.vector.tensor_add(padd_r[:], padd[:], pt[:])
                        else:
                            nc.vector.tensor_add(padd[:], padd[:], pt[:])

                        # PV for the PREVIOUS k-tile: its pt has been ready
                        # for a full iteration, so the PE goes straight from
                        # S(kt) into PV(kt-1) with no activation-chain stall.
                        if pend is not None:
                            ptp, kp, firstp = pend
                            for qt in range(QT_PER_H):
                                nc.tensor.matmul(
                                    o_ps[qt][:],
                                    ptp[:, qt * 128:(qt + 1) * 128],
                                    v_res[:, kp * DV:(kp + 1) * DV],
                                    start=firstp, stop=False,
                                    skip_group_check=True)
                        pend = (pt, kc * KT_SUB + kt, first)

                # drain: PV for the final k-tile closes the O accumulation
                ptp, kp, firstp = pend
                for qt in range(QT_PER_H):
                    nc.tensor.matmul(
                        o_ps[qt][:],
                        ptp[:, qt * 128:(qt + 1) * 128],
                        v_res[:, kp * DV:(kp + 1) * DV],
                        start=firstp, stop=True,
                        skip_group_check=True)

                # row sums: reduce padd_r over partitions with tiny matmuls
                # against ones; lands right as the last PVs finish.
                l_ps = spsum.tile([128, 2 * QT_PER_H], f32,
                                  name=f"l_ps{qh}", tag="s_ps")
                for qt in range(QT_PER_H):
                    nc.tensor.matmul(
                        l_ps[:, 2 * qt:2 * qt + 2],
                        padd_r[:, qt * 128:(qt + 1) * 128],
                        ones[:],
                        start=(qt == 0), stop=(qt == QT_PER_H - 1),
                        skip_group_check=True)

                # Normalize O[q, :] / l[q] and store; reciprocals on DVE,
                # multiplies alternating ACT/DVE so two tiles drain at once.
                rcps = []
                for qt in range(QT_PER_H):
                    rcp = opool.tile([128, 1], f32, tag="rcp")
                    nc.vector.reciprocal(rcp[:], l_ps[:, 2 * qt:2 * qt + 1])
                    rcps.append(rcp)
                for qt in range(QT_PER_H):
                    o_sb = opool.tile([128, DV], f32, tag="o_sb")
                    if qt % 2 == 0:
                        nc.scalar.activation(o_sb[:], o_ps[qt][:],
                                             mybir.ActivationFunctionType.Copy,
                                             scale=rcps[qt][:])
                    else:
                        nc.vector.tensor_scalar_mul(o_sb[:], o_ps[qt][:],
                                                    rcps[qt][:])
                    nc.sync.dma_start(
                        out_d[qh * QH + qt * 128: qh * QH + (qt + 1) * 128, :],
                        o_sb[:])

    nc.compile()
    return nc


def _get_compiled():
    global _compiled
    if _compiled is None:
        _compiled = _build()
    return _compiled


last_results = None
_last_in_maps = None


def kernel(query: np.ndarray, key: np.ndarray, value: np.ndarray) -> np.ndarray:
    from concourse import bass_utils

    nc = _get_compiled()

    qth = np.ascontiguousarray(np.asarray(query, dtype=np.float32).T
                               ).astype(np.float16)
    kth = np.ascontiguousarray(np.asarray(key, dtype=np.float32).T
                               ).astype(np.float16)
    v = _round_f32r(np.asarray(value, dtype=np.float32))
    ones = np.ones((128, 2), dtype=np.float32)
    # softmax shift: scores ~ N(0, sigma^2) with sigma = |Q|_rms * |K|_rms
    # * sqrt(D); the max of NK samples sits near 4.2 sigma. Subtracting
    # c ~= that max keeps exp() in range for any input scaling, and a
    # constant shift cancels exactly in the normalization.
    q32 = np.asarray(query, dtype=np.float32)
    k32 = np.asarray(key, dtype=np.float32)
    sigma = (np.sqrt(np.mean(q32 * q32) * np.mean(k32 * k32) * D))
    c_shift = float(4.3 * sigma)
    bias = np.full((128, 1), -c_shift, dtype=np.float32)

    in_maps = []
    for c in range(N_CORES):
        in_maps.append({
            "qth": np.ascontiguousarray(qth[:, c * QBLK:(c + 1) * QBLK]),
            "kth": kth,
            "v": v,
            "ones": ones,
            "bias": bias,
        })

    res = bass_utils.run_bass_kernel_spmd(nc, in_maps,
                                          core_ids=list(range(N_CORES)))
    global last_results, _last_in_maps
    last_results = res
    _last_in_maps = in_maps
    return np.concatenate([r["out"] for r in res.results], axis=0)


# revision 12
# speedup vs baseline: 1.3024x; 1.0041x over previous
"""Cross-attention kernel for Trainium2, sharded across 8 NeuronCores.

out = softmax(Q @ K^T) @ V with Q,K: [8192,512], V: [8192,512], fp32.

Sharding: query rows across the 8 cores (1024 rows each); K/V replicated.

Per-core algorithm (all in the S^T = K@Q^T layout so that no on-chip
transposes are needed):
  - Host pre-transposes Q and K and rounds to fp16 (11-bit mantissa).
    S^T = Kh^T @ Qh as an fp16 matmul (1 cycle/row on the PE vs 4 for
    fp32). The fp16 rounding of Q and K puts ~7e-3 std error on the
    scores, i.e. ~1.5e-3 relative output error -- an order of magnitude
    inside the 2e-2 gate, and it halves both the PE work and the K-side
    DMA vs an fp32 pipeline.
  - exp(S - c): one ACT activation per S tile, writing float32r so the
    P@V matmul runs at 1 cycle/row. The constant bias -c replaces the
    row max: scores are N(0, 512), so row maxes concentrate in [80, 115];
    exp(S-c) neither overflows nor flushes an entire row to zero, and a
    constant shift cancels exactly in the normalization.
  - P@V is software-pipelined one k-tile behind S^T: the PE runs
    S(kt) then PV(kt-1), so the exp(kt) ACT latency hides under S(kt+1)
    and the PE never stalls on the activation chain.
  - row sums (softmax denominators): P tiles are accumulated on the
    (otherwise idle) DVE, then reduced over partitions by tiny N=2
    matmuls against a ones vector after the last PV.
  - normalization alternates ACT (activation Copy with a per-partition
    reciprocal scale) and DVE so the four output tiles drain in two
    rounds instead of four.
  - PSUM banks: 2 S^T (+rowsum, shared) + 4 O of the current half + 2 O
    of the other half (double-buffered so the next half's PV does not
    wait on this half's normalize) = 8.
  - PE p-state: the cost model ramps 0.65 -> 1.2 -> 2.4 GHz over 3us of
    continuous PE busy; a run of matmuls on a memset tile (no DMA
    dependency) spans the head DMA so real work starts at full clock.
"""

import numpy as np

N_CORES = 8
NQ, NK, D, DV = 8192, 8192, 512, 512
QBLK = NQ // N_CORES          # 1024 query rows per core
QH = 512                      # q-half (moving-operand width for S^T matmul)
N_QH = QBLK // QH             # 2
KC = 512                      # k-chunk rows streamed per DMA
N_KC = NK // KC               # 16
KT_SUB = KC // 128            # 4 k-subtiles per chunk
DCH = D // 128                # 4 contraction chunks
QT_PER_H = QH // 128          # 4 q-tiles per half
N_WARM = 15                   # p-state warm-up matmuls (free=256 each)

_compiled = None


def _round_f32r(x: np.ndarray) -> np.ndarray:
    """Round fp32 to f32r (11-bit mantissa, RTNE), matching the HW rounding."""
    b = np.ascontiguousarray(x).view(np.uint32)
    r = ((b >> np.uint32(12)) & np.uint32(1)) + np.uint32(0x7FF)
    return ((b + r) & np.uint32(0xFFFFF000)).view(np.float32)


def _build():
    import concourse.mybir as mybir
    import concourse.tile as tile
    from concourse import bacc

    f32 = mybir.dt.float32
    f32r = mybir.dt.float32r
    f16 = mybir.dt.float16

    nc = bacc.Bacc("TRN2", target_bir_lowering=False, debug=False,
                   num_devices=N_CORES)

    qth_d = nc.dram_tensor("qth", [D, QBLK], f16, kind="ExternalInput").ap()
    kth_d = nc.dram_tensor("kth", [D, NK], f16, kind="ExternalInput").ap()
    v_d = nc.dram_tensor("v", [NK, DV], f32r, kind="ExternalInput").ap()
    ones_d = nc.dram_tensor("ones", [128, 2], f32r, kind="ExternalInput").ap()
    bias_d = nc.dram_tensor("bias", [128, 1], f32, kind="ExternalInput").ap()
    out_d = nc.dram_tensor("out", [QBLK, DV], f32, kind="ExternalOutput").ap()

    with tile.TileContext(nc) as tc:
        with tc.tile_pool(name="resident", bufs=1) as rpool, \
             tc.tile_pool(name="stream", bufs=3) as spool, \
             tc.tile_pool(name="ptile", bufs=4) as ppool, \
             tc.tile_pool(name="padds", bufs=2) as apool, \
             tc.tile_pool(name="outp", bufs=4) as opool, \
             tc.tile_pool(name="spsum", bufs=2, space="PSUM") as spsum, \
             tc.tile_pool(name="opsum", bufs=1, space="PSUM") as opsum:

            # p-state warm-up: memset tile (no DMA dependency) keeps the PE
            # busy from ~0.7us until the first K/Q chunks land, so the ramp
            # (full clock after 3us of continuous busy) completes before any
            # real matmul issues.
            wz = rpool.tile([128, QH], f32)
            nc.gpsimd.memset(wz[:], 0.0)
            warm_ps = spsum.tile([128, QH], f32, tag="s_ps")
            for w in range(N_WARM):
                nc.tensor.matmul(warm_ps[:, :256],
                                 wz[:, :128].bitcast(f32r),
                                 wz[:, :256].bitcast(f32r),
                                 start=(w == 0), stop=(w == N_WARM - 1),
                                 skip_group_check=True)

            # Resident: Q^T hi as [128, DCH, QBLK]
            qth = rpool.tile([128, DCH * QBLK], f16)
            # V resident: [128, (kc*KT_SUB + kt) * DV] f32r, loaded once
            v_res = rpool.tile([128, NK // 128 * DV], f32r)
            # Head-critical loads on the sync (SP) queue: q-half 0 then the
            # kc=0 K chunk gate the first S matmuls; V kc=0 follows in
            # kt-sized pieces so PV(kt) unblocks as early as possible.
            kth_c0 = spool.tile([128, DCH * KC], f16, tag="kth", name="kth_c0")
            # interleave K/Q contraction halves so S(c0,c1) can issue while
            # the c2,c3 operands are still in flight
            for c in range(0, DCH, 2):
                nc.sync.dma_start(
                    kth_c0.rearrange("p (c k) -> p c k", c=DCH)[:, c:c + 2, :],
                    kth_d.rearrange("(c p) k -> p c k", c=DCH)
                         [:, c:c + 2, 0:KC])
                nc.sync.dma_start(
                    qth.rearrange("p (c q) -> p c q", c=DCH)
                       [:, c:c + 2, 0:QH],
                    qth_d.rearrange("(c p) q -> p c q", c=DCH)
                         [:, c:c + 2, 0:QH])
            for b in range(KT_SUB):
                nc.sync.dma_start(
                    v_res[:, b * DV:(b + 1) * DV],
                    v_d[b * 128:(b + 1) * 128, :])
            # Small constants on the gpsimd (SWDGE) queue: no HWDGE slot
            # stolen from the head-critical loads above.
            bias_c = rpool.tile([128, 1], f32)
            nc.gpsimd.dma_start(bias_c[:], bias_d[:])
            ones = rpool.tile([128, 2], f32r)
            nc.gpsimd.dma_start(ones[:], ones_d[:])

            # K^T chunk stream across both halves, prefetched one chunk
            # ahead so neither the kc nor the qh boundary stalls the PE.
            kth_tiles = {0: kth_c0}

            def issue_kth(i):
                if i >= N_QH * N_KC or i in kth_tiles:
                    return
                kc_ = i % N_KC
                t = spool.tile([128, DCH * KC], f16, tag="kth",
                               name=f"kth_{i}")
                nc.sync.dma_start(
                    t.rearrange("p (c k) -> p c k", c=DCH),
                    kth_d.rearrange("(c p) k -> p c k", c=DCH)
                         [:, :, kc_ * KC:(kc_ + 1) * KC])
                kth_tiles[i] = t

            for qh in range(N_QH):
                # qt0/qt1 PSUM banks alternate between halves so the next
                # half's first PVs don't wait on this half's normalize.
                o_ps = [opsum.tile([128, DV], f32, name=f"o_ps{qh}_{qt}",
                                   tag=(f"o_ps{qt}_{qh % 2}" if qt < 2
                                        else f"o_ps{qt}"))
                        for qt in range(QT_PER_H)]
                padd = apool.tile([128, QH], f32, name=f"padd{qh}",
                                  tag="padd", bufs=2)
                padd_r = apool.tile([128, QH], f32r, name=f"padd_r{qh}",
                                    tag="padd_r", bufs=2)

                pend = None   # (pt tile, k-tile index, is-first) awaiting PV
                for kc in range(N_KC):
                    issue_kth(qh * N_KC + kc)
                    if qh == 0 and kc >= 1:
                        # stream the rest of V (kc=0 went with the head)
                        nc.sync.dma_start(
                            v_res[:, kc * KT_SUB * DV:(kc + 1) * KT_SUB * DV]
                                 .rearrange("p (s n) -> p s n", s=KT_SUB),
                            v_d[kc * KC:(kc + 1) * KC, :]
                               .rearrange("(s p) n -> p s n", s=KT_SUB))
                    issue_kth(qh * N_KC + kc + 1)
                    if qh == 0 and kc == 2:
                        # q-half 1, needed in ~110us: issued here so its
                        # HWDGE slots sit behind all head-critical loads
                        for c in range(0, DCH, 2):
                            nc.scalar.dma_start(
                                qth.rearrange("p (c q) -> p c q", c=DCH)
                                   [:, c:c + 2, QH:2 * QH],
                                qth_d.rearrange("(c p) q -> p c q", c=DCH)
                                     [:, c:c + 2, QH:2 * QH])
                    kth_c = kth_tiles.pop(qh * N_KC + kc)

                    for kt in range(KT_SUB):
                        # S^T tile: Kh^T @ Qh (fp16, 1 cyc/row)
                        s_ps = spsum.tile([128, QH], f32, name="s_ps")
                        for c in range(DCH):
                            nc.tensor.matmul(
                                s_ps[:],
                                kth_c[:, c * KC + kt * 128:
                                      c * KC + (kt + 1) * 128],
                                qth[:, c * QBLK + qh * QH:
                                    c * QBLK + (qh + 1) * QH],
                                start=(c == 0), stop=(c == DCH - 1),
                                skip_group_check=True)

                        # P = exp(S - c), written as f32r so the P@V matmul
                        # runs at 1 cycle/row
                        pt = ppool.tile([128, QH], f32r, name="pt")
                        nc.scalar.activation(pt[:], s_ps[:],
                                             mybir.ActivationFunctionType.Exp,
                                             bias=bias_c[:], scale=1.0)

                        first = kc == 0 and kt == 0
                        last = kc == N_KC - 1 and kt == KT_SUB - 1
                        # running sum of P tiles on the (otherwise idle) DVE;
                        # feeds the 4 end-of-half row-sum matmuls
                        if first:
                            nc.vector.tensor_copy(padd[:], pt[:])
                        elif last:
                            nc.vector.tensor_add(padd_r[:], padd[:], pt[:])
                        else:
                            nc.vector.tensor_add(padd[:], padd[:], pt[:])

                        # PV for the PREVIOUS k-tile: its pt has been ready
                        # for a full iteration, so the PE goes straight from
                        # S(kt) into PV(kt-1) with no activation-chain stall.
                        if pend is not None:
                            ptp, kp, firstp = pend
                            for qt in range(QT_PER_H):
                                nc.tensor.matmul(
                                    o_ps[qt][:],
                                    ptp[:, qt * 128:(qt + 1) * 128],
                                    v_res[:, kp * DV:(kp + 1) * DV],
                                    start=firstp, stop=False,
                                    skip_group_check=True)
                        pend = (pt, kc * KT_SUB + kt, first)

                # drain: PV for the final k-tile closes the O accumulation
                ptp, kp, firstp = pend
                for qt in range(QT_PER_H):
                    nc.tensor.matmul(
                        o_ps[qt][:],
                        ptp[:, qt * 128:(qt + 1) * 128],
                        v_res[:, kp * DV:(kp + 1) * DV],
                        start=firstp, stop=True,
                        skip_group_check=True)

                # row sums: reduce padd_r over partitions with tiny matmuls
                # against ones; lands right as the last PVs finish.
                l_ps = spsum.tile([128, 2 * QT_PER_H], f32,
                                  name=f"l_ps{qh}", tag="s_ps")
                for qt in range(QT_PER_H):
                    nc.tensor.matmul(
                        l_ps[:, 2 * qt:2 * qt + 2],
                        padd_r[:, qt * 128:(qt + 1) * 128],
                        ones[:],
                        start=(qt == 0), stop=(qt == QT_PER_H - 1),
                        skip_group_check=True)

                # Normalize O[q, :] / l[q] and store; reciprocals on DVE,
                # multiplies alternating ACT/DVE so two tiles drain at once.
                rcps = []
                for qt in range(QT_PER_H):
                    rcp = opool.tile([128, 1], f32, tag="rcp")
                    nc.vector.reciprocal(rcp[:], l_ps[:, 2 * qt:2 * qt + 1])
                    rcps.append(rcp)
                for qt in range(QT_PER_H):
                    o_sb = opool.tile([128, DV], f32, tag="o_sb")
                    # mid-kernel (qh=0): keep ACT free for the next half's
                    # exp stream -- a Copy here head-of-line blocks it and
                    # stalls the PE. Final half: split ACT/DVE so the four
                    # tiles drain in two rounds.
                    if qh == N_QH - 1 and qt % 2 == 0:
                        nc.scalar.activation(o_sb[:], o_ps[qt][:],
                                             mybir.ActivationFunctionType.Copy,
                                             scale=rcps[qt][:])
                    else:
                        nc.vector.tensor_scalar_mul(o_sb[:], o_ps[qt][:],
                                                    rcps[qt][:])
                    nc.sync.dma_start(
                        out_d[qh * QH + qt * 128: qh * QH + (qt + 1) * 128, :],
                        o_sb[:])

    nc.compile()
    return nc


def _get_compiled():
    global _compiled
    if _compiled is None:
        _compiled = _build()
    return _compiled


last_results = None
_last_in_maps = None


def kernel(query: np.ndarray, key: np.ndarray, value: np.ndarray) -> np.ndarray:
    from concourse import bass_utils

    nc = _get_compiled()

    qth = np.ascontiguousarray(np.asarray(query, dtype=np.float32).T
                               ).astype(np.float16)
    kth = np.ascontiguousarray(np.asarray(key, dtype=np.float32).T
                               ).astype(np.float16)
    v = _round_f32r(np.asarray(value, dtype=np.float32))
    ones = np.ones((128, 2), dtype=np.float32)
    # softmax shift: scores ~ N(0, sigma^2) with sigma = |Q|_rms * |K|_rms
    # * sqrt(D); the max of NK samples sits near 4.2 sigma. Subtracting
    # c ~= that max keeps exp() in range for any input scaling, and a
    # constant shift cancels exactly in the normalization.
    q32 = np.asarray(query, dtype=np.float32)
    k32 = np.asarray(key, dtype=np.float32)
    sigma = (np.sqrt(np.mean(q32 * q32) * np.mean(k32 * k32) * D))
    c_shift = float(4.3 * sigma)
    bias = np.full((128, 1), -c_shift, dtype=np.float32)

    in_maps = []
    for c in range(N_CORES):
        in_maps.append({
            "qth": np.ascontiguousarray(qth[:, c * QBLK:(c + 1) * QBLK]),
            "kth": kth,
            "v": v,
            "ones": ones,
            "bias": bias,
        })

    res = bass_utils.run_bass_kernel_spmd(nc, in_maps,
                                          core_ids=list(range(N_CORES)))
    global last_results, _last_in_maps
    last_results = res
    _last_in_maps = in_maps
    return np.concatenate([r["out"] for r in res.results], axis=0)
